# revision 28
# baseline (speedup 1.0000x reference)
"""Fused biased-softmax attention (nn_Attention_55576876810478) on 8 TRN2 NeuronCores.

Sharding: 2-D (batch x head-group).  Core c = (b, hg) with b = c//2, hg = c%2
owns batch b and heads 4*hg .. 4*hg+3 (4 heads x 32 ch = 128 columns of every
projection).  Each core computes its batch's q/k/v/gate projections for its 4
heads, the biased softmax attention, the sigmoid gate, the per-head softmax
normalization, and its 128 rows of the output projection -- producing a
partial [Q, D] output per core.  The host sums each batch's two partials and
adds bo.

Key on-chip structure (per core):
  * projections: stationaries [256, 128] -> psum [128(h,c), 512-token chunks];
    the (h, c) row layout IS the 4-row-band layout the packed score matmuls
    need, so there are no relayout DMAs for q/k at all.
  * v is projected with kvx^T chunks as the *stationary* (out = v[k, (h,c)]),
    which lands v directly in the [k%128, ktile, c] layout the PV matmul
    wants -- no transposes.
  * scores are computed transposed, S^T[k, q], 4 heads packed in the 4 PE
    row-bands (concurrent matmuls).  bias_pair + bias_mask arrive raw (bf16)
    and are accumulated into the score PSUM with an identity matmul before a
    single exp -- no separate bias multiply pass.
  * exp runs on [128, 1024] 2-bank psum tiles (amortizes ACT overhead).
  * PV uses a [v | ones] stationary (row 32 of each accumulator = softmax
    denominator) with two heads col-packed per PSUM bank (bases 0 / 64).
  * denominators are transposed to per-partition columns with a strided
    SBUF->SBUF DMA (den4[p, c] covers q = 512 jq + 4p + j4), reciprocated in
    one cheap [128, 16] DVE op, and applied as per-partition scalars while
    accumulating the four per-head output-projection matmuls on DVE
    (ts/scalar_tensor_tensor chain), so no partition broadcast is needed.
  * input DMAs are split across both HWDGE rings (sync: q-side, scalar:
    kv-side); the 8 MB of bias tiles stream on the gpsimd SWDGE queue so
    they never serialize behind the HWDGE traffic.
"""

import math

import ml_dtypes
import numpy as np

B, Q, KL, D, H, C = 4, 1024, 1024, 256, 8, 32
NCORES = 8
NKT = KL // 128        # 8 k-tiles
NCH = Q // 512         # 2 512-token chunks

_BF16 = ml_dtypes.bfloat16
_CACHE = {}


def _build_nc():
    import concourse.bass as bass  # noqa: F401
    import concourse.mybir as mybir
    import concourse.tile as tile
    from concourse.bacc import Bacc

    bf16 = mybir.dt.bfloat16
    f32 = mybir.dt.float32
    AF = mybir.ActivationFunctionType
    ALU = mybir.AluOpType

    nc = Bacc(None, target_bir_lowering=False)

    qxT_d = nc.dram_tensor("qxT", [2, 128, Q], bf16, kind="ExternalInput")
    kvxT_d = nc.dram_tensor("kvxT", [2, 128, KL], bf16, kind="ExternalInput")
    wq4_d = nc.dram_tensor("wq4", [2, 128, 128], bf16, kind="ExternalInput")
    wg4_d = nc.dram_tensor("wg4", [2, 128, 128], bf16, kind="ExternalInput")
    wk4_d = nc.dram_tensor("wk4", [2, 128, 128], bf16, kind="ExternalInput")
    wv4_d = nc.dram_tensor("wv4", [2, 128, 128], bf16, kind="ExternalInput")
    bg4_d = nc.dram_tensor("bg4", [128, 1], f32, kind="ExternalInput")
    biasT_d = nc.dram_tensor("biasT", [4, 4, 128, 2048], bf16,
                             kind="ExternalInput")
    ident_d = nc.dram_tensor("ident", [128, 128], bf16, kind="ExternalInput")
    woh_d = nc.dram_tensor("woh", [32, 4 * D], bf16, kind="ExternalInput")
    out_d = nc.dram_tensor("out", [Q, D], f32, kind="ExternalOutput")
    out_r = out_d[:, :].rearrange("(t p j) d -> t j p d", p=128, j=4)

    with tile.TileContext(nc) as tc:
        with (
            tc.tile_pool(name="const", bufs=1) as const,
            tc.tile_pool(name="biasp", bufs=1) as biasp,
            tc.tile_pool(name="proj", bufs=1) as proj,
            tc.tile_pool(name="pp", bufs=12) as pp,
            tc.tile_pool(name="dnp", bufs=2) as dnp,
            tc.tile_pool(name="outp", bufs=3) as outp,
        ):
            # ---------------- input DMAs ----------------
            qxT = const.tile([128, 2, Q], bf16)
            kvxT = const.tile([128, 2, KL], bf16)
            wq4 = const.tile([128, 2, 128], bf16)
            wg4 = const.tile([128, 2, 128], bf16)
            wk4 = const.tile([128, 2, 128], bf16)
            wv4 = const.tile([128, 2, 128], bf16)
            for eng, grp in ((nc.sync, ((qxT, qxT_d), (wq4, wq4_d))),
                             (nc.scalar, ((kvxT, kvxT_d), (wk4, wk4_d),
                                          (wg4, wg4_d), (wv4, wv4_d)))):
                for sb, dr in grp:
                    for dc in range(2):
                        eng.dma_start(sb[:, dc, :], dr[dc])
            bg4 = const.tile([128, 1], f32)
            nc.scalar.dma_start(bg4, bg4_d[:, :])
            ident = const.tile([128, 128], bf16)
            nc.sync.dma_start(ident, ident_d[:, :])
            woh = const.tile([32, 4, D], bf16)
            nc.sync.dma_start(woh, woh_d[:, :])
            # gate the bias stream: bias descriptors are only generated
            # after the input tensors have landed, so the input DMAs are not
            # starved of SDMA bandwidth by the 8 MB bias stream
            gate_sc = const.tile([1, 64], bf16)
            nc.gpsimd.dma_start(gate_sc, qxT[0:1, 1, 0:64])
            nc.gpsimd.dma_start(gate_sc, kvxT[0:1, 1, 0:64])
            # bias tiles: bp[j][ktpair] = [128, (kt2, q)]; issued in
            # consumption order (ktpair-major)
            bp = [[None] * 4 for _ in range(4)]
            for ktp in range(4):
                for j in range(4):
                    t = biasp.tile([128, 2048], bf16, tag=f"bp{j}_{ktp}",
                                   name=f"bp{j}_{ktp}")
                    nc.gpsimd.dma_start(t, biasT_d[j, ktp])
                    bp[j][ktp] = t

            # ---------------- persistent intermediates ----------------
            qTb = proj.tile([128, Q], bf16)      # q^T, rows (h, c)
            kTb = proj.tile([128, KL], bf16)     # k^T, rows (h, c)
            gT = proj.tile([128, Q], bf16)       # sigmoid gate, rows (h, c)
            gT33 = proj.tile([128, 4, Q], bf16)  # per-head gate + ones row 32
            vones = proj.tile([128, 4, NKT, 33], bf16)  # [k%128, h, kt, c|1]
            # per-head gated O^T + den row 32, all at base partition 0
            odn = [proj.tile([33, Q], bf16, name=f"odn{j}") for j in range(4)]
            den4 = proj.tile([128, 32], bf16)    # den4[p, 8h+4jq+j4]
            recip4 = proj.tile([128, 32], f32)

            nc.vector.memset(vones, 1.0)
            nc.vector.memset(gT33[32:33, :, :], 1.0)
            zst = proj.tile([128, 33], bf16)
            nc.vector.memset(zst, 0.0)

            # ---------------- projections ----------------
            with tc.tile_pool(name="ps_pj", bufs=3, space="PSUM") as ps_pj:
                for ch in range(NCH):
                    sl = slice(ch * 512, (ch + 1) * 512)
                    q_ps = ps_pj.tile([128, 512], f32, tag="pj")
                    for dc in range(2):
                        nc.tensor.matmul(q_ps, wq4[:, dc, :], qxT[:, dc, sl],
                                         start=dc == 0, stop=dc == 1)
                    nc.vector.tensor_copy(qTb[:, sl], q_ps)
                    k_ps = ps_pj.tile([128, 512], f32, tag="pj")
                    for dc in range(2):
                        nc.tensor.matmul(k_ps, wk4[:, dc, :], kvxT[:, dc, sl],
                                         start=dc == 0, stop=dc == 1)
                    nc.vector.tensor_copy(kTb[:, sl], k_ps)
                    g_ps = ps_pj.tile([128, 512], f32, tag="pj")
                    for dc in range(2):
                        nc.tensor.matmul(g_ps, wg4[:, dc, :], qxT[:, dc, sl],
                                         start=dc == 0, stop=dc == 1)
                    # sigmoid(x) = 0.5*tanh(0.5x) + 0.5 (tanh shares the exp
                    # ACT table set)
                    nc.scalar.activation(gT[:, sl], g_ps, AF.Tanh,
                                         bias=bg4, scale=0.5)
                    nc.vector.tensor_scalar(gT[:, sl], gT[:, sl], 0.5, 0.5,
                                            op0=ALU.mult, op1=ALU.add)
                # v projected with kvx^T as stationary: v_ps[k, (h, c)]
                for kt in range(NKT):
                    ksl = slice(kt * 128, (kt + 1) * 128)
                    v_ps = ps_pj.tile([128, 128], f32, tag="vps")
                    for dc in range(2):
                        nc.tensor.matmul(v_ps, kvxT[:, dc, ksl], wv4[:, dc, :],
                                         start=dc == 0, stop=dc == 1)
                    nc.vector.tensor_copy(vones[:, :, kt, 0:C], v_ps)
            # per-head gate rows -> gT33 (ones row 32 already set)
            for j in range(4):
                nc.sync.dma_start(gT33[0:C, j, :], gT[32 * j:32 * (j + 1), :])

            # ---------------- attention ----------------
            with (
                tc.tile_pool(name="ps_s", bufs=3, space="PSUM") as ps_s,
                tc.tile_pool(name="ps_pv", bufs=2, space="PSUM") as ps_pv,
            ):
                for jq in range(2):
                    qsl = slice(jq * 512, (jq + 1) * 512)
                    pvA = ps_pv.tile([128, 512], f32, tag="pv",
                                     name=f"pvA_{jq}")
                    pvB = ps_pv.tile([128, 512], f32, tag="pv",
                                     name=f"pvB_{jq}")
                    inited = [False]

                    def emit_pv(kt, prawA, prawB):
                        if not inited[0]:
                            # zero-init the upper (base-64) col-tile regions:
                            # their accumulating matmuls use start=False (the
                            # lower tile's start clears the whole bank's
                            # has_written bits on HW).  Deferred here so it
                            # doesn't head-block this block's score matmuls
                            # on the previous block's psum slot release.
                            for pv in (pvA, pvB):
                                nc.tensor.matmul(pv[64:97, :], zst,
                                                 qTb[:, qsl],
                                                 start=True, stop=False,
                                                 tile_position=(0, 64),
                                                 skip_group_check=True)
                            inited[0] = True
                        for pv, praw, jlo in ((pvA, prawA, 0), (pvB, prawB, 2)):
                            nc.tensor.matmul(
                                pv[0:33, :], vones[:, jlo, kt, :],
                                praw[:, 0:512],
                                start=kt == 0, stop=kt == NKT - 1,
                                tile_position=(0, 0))
                        for pv, praw, jlo in ((pvA, prawA, 0), (pvB, prawB, 2)):
                            nc.tensor.matmul(
                                pv[64:97, :], vones[:, jlo + 1, kt, :],
                                praw[:, 512:1024],
                                start=False, stop=kt == NKT - 1,
                                tile_position=(0, 64),
                                skip_group_check=True)

                    pending = []
                    for kt in range(NKT):
                        ktp, kt2 = kt // 2, kt % 2
                        ksl = slice(kt * 128, (kt + 1) * 128)
                        sA = ps_s.tile([128, 1024], f32, tag="s",
                                       name=f"sA_{jq}_{kt}")
                        sB = ps_s.tile([128, 1024], f32, tag="s",
                                       name=f"sB_{jq}_{kt}")
                        # per s-tile: 2 packed score matmuls then its 2
                        # bias (identity) matmuls, so each exp unblocks as
                        # soon as its own tile's matmuls retire
                        for s_t, jlo in ((sA, 0), (sB, 2)):
                            for j in (jlo, jlo + 1):
                                half = slice((j % 2) * 512,
                                             (j % 2) * 512 + 512)
                                nc.tensor.matmul(
                                    s_t[:, half],
                                    kTb[32 * j:32 * (j + 1), ksl],
                                    qTb[32 * j:32 * (j + 1), qsl],
                                    start=True, stop=False,
                                    tile_position=(32 * j, 0))
                            for j in (jlo, jlo + 1):
                                half = slice((j % 2) * 512,
                                             (j % 2) * 512 + 512)
                                bsl = slice(kt2 * 1024 + jq * 512,
                                            kt2 * 1024 + jq * 512 + 512)
                                nc.tensor.matmul(
                                    s_t[:, half], ident, bp[j][ktp][:, bsl],
                                    start=False, stop=True)
                        prawA = pp.tile([128, 1024], bf16, tag="praw",
                                        name=f"prawA_{jq}_{kt}")
                        nc.scalar.activation(prawA, sA, AF.Exp)
                        prawB = pp.tile([128, 1024], bf16, tag="praw",
                                        name=f"prawB_{jq}_{kt}")
                        nc.scalar.activation(prawB, sB, AF.Exp)
                        pending.append((kt, prawA, prawB))
                        # deep PV lag: keeps the next block's score matmuls
                        # ahead of pv-slot waits in the PE queue
                        if len(pending) > 5:
                            emit_pv(*pending.pop(0))
                    for args in pending:
                        emit_pv(*args)

                    # gate + extract denominators, normalize, project out
                    # upper-half STT first: it waits on the bank's LAST
                    # matmul (h1/h3 kt=7), so the later lower-half read can't
                    # collide with an in-flight PE write to the same bank
                    for pv, jlo in ((pvA, 0), (pvB, 2)):
                        nc.vector.scalar_tensor_tensor(
                            odn[jlo + 1][0:33, qsl], pv[64:97, :], 1.0,
                            gT33[0:33, jlo + 1, qsl],
                            op0=ALU.mult, op1=ALU.mult)
                        nc.vector.scalar_tensor_tensor(
                            odn[jlo][0:33, qsl], pv[0:33, :], 1.0,
                            gT33[0:33, jlo, qsl],
                            op0=ALU.mult, op1=ALU.mult)
                    # denominators -> per-partition layout: den4[p, c] with
                    # c = 8h + 4jq + j4 covering q = 512 jq + 4p + j4
                    for j in range(4):
                        nc.gpsimd.dma_start(den4[:, 8 * j + 4 * jq:
                                                 8 * j + 4 * jq + 4],
                                            odn[j][32:33, qsl])
                    rsl = den4[:, :].rearrange("p (h t j) -> p h t j", h=4, t=2)
                    osl = recip4[:, :].rearrange("p (h t j) -> p h t j", h=4, t=2)
                    nc.vector.reciprocal(osl[:, :, jq, :], rsl[:, :, jq, :])
                    # per-head output projection + recip-scaled eviction
                    for j4 in range(4):
                        fos = []
                        for pi, jlo in ((0, 0), (1, 2)):
                            fo = ps_pv.tile([128, 512], f32, tag="pv",
                                            name=f"fo{pi}_{jq}_{j4}")
                            for ih in range(2):
                                j = jlo + ih
                                og = odn[j][0:C, qsl].rearrange(
                                    "c (p j) -> c j p", j=4)
                                nc.tensor.matmul(fo[:, ih * D:(ih + 1) * D],
                                                 og[:, j4, :], woh[:, j, :],
                                                 start=True, stop=True)
                            fos.append(fo)
                        # evict: reverse order so the first read waits on the
                        # bank's last matmul
                        acc = None
                        for j in (3, 2, 1, 0):
                            fo = fos[j // 2]
                            half = slice((j % 2) * D, (j % 2) * D + D)
                            rc = recip4[:, 8 * j + 4 * jq + j4:
                                        8 * j + 4 * jq + j4 + 1]
                            ot = outp.tile([128, D], f32, tag="ot",
                                           name=f"ot_{jq}_{j4}_{j}")
                            if acc is None:
                                nc.vector.tensor_scalar(ot, fo[:, half], rc,
                                                        None, op0=ALU.mult)
                            else:
                                nc.vector.scalar_tensor_tensor(
                                    ot, fo[:, half], rc, acc,
                                    op0=ALU.mult, op1=ALU.add)
                            acc = ot
                        nc.sync.dma_start(out_r[jq, j4], acc)

    nc.finalize()
    return nc


def _get_nc():
    if "nc" not in _CACHE:
        _CACHE["nc"] = _build_nc()
    return _CACHE["nc"]


def _prep(inputs):
    q_x = np.asarray(inputs["q_x"], np.float32)
    kv_x = np.asarray(inputs["kv_x"], np.float32)
    bias_mask = np.asarray(inputs["bias_mask"], np.float32)
    bias_pair = np.asarray(inputs["bias_pair"], np.float32)
    wq = np.asarray(inputs["wq"], np.float32)
    wk = np.asarray(inputs["wk"], np.float32)
    wv = np.asarray(inputs["wv"], np.float32)
    wg = np.asarray(inputs["wg"], np.float32)
    bg = np.asarray(inputs["bg"], np.float32)
    wo = np.asarray(inputs["wo"], np.float32)

    sc = 1.0 / math.sqrt(C)
    ident = np.eye(128, dtype=_BF16)
    bmk = bias_mask.reshape(B, KL)

    in_maps = []
    for core in range(NCORES):
        b, hg = core // 2, core % 2
        hsl = slice(hg * 128, (hg + 1) * 128)
        qxT = np.ascontiguousarray(q_x[b].T).astype(_BF16).reshape(2, 128, Q)
        kvxT = np.ascontiguousarray(kv_x[b].T).astype(_BF16).reshape(2, 128, KL)
        # bias: [4h, K, Q] -> [4h, ktpair, p, kt2, q] -> [4, 4, 128, 2048]
        bT = (bias_pair[b, 4 * hg:4 * hg + 4].transpose(0, 2, 1)
              + bmk[b][None, :, None])
        bT = bT.reshape(4, 4, 2, 128, Q).transpose(0, 1, 3, 2, 4)
        bT = np.ascontiguousarray(bT).astype(_BF16).reshape(4, 4, 128, 2048)

        in_maps.append({
            "qxT": qxT,
            "kvxT": kvxT,
            "wq4": np.ascontiguousarray(wq[:, hsl] * sc).astype(_BF16).reshape(2, 128, 128),
            "wg4": np.ascontiguousarray(wg[:, hsl]).astype(_BF16).reshape(2, 128, 128),
            "wk4": np.ascontiguousarray(wk[:, hsl]).astype(_BF16).reshape(2, 128, 128),
            "wv4": np.ascontiguousarray(wv[:, hsl]).astype(_BF16).reshape(2, 128, 128),
            "bg4": (0.5 * bg[hsl]).astype(np.float32).reshape(128, 1),
            "biasT": bT,
            "ident": ident,
            "woh": np.ascontiguousarray(
                wo[hsl].reshape(4, C, D).transpose(1, 0, 2)
            ).astype(_BF16).reshape(C, 4 * D),
        })
    return in_maps


def _run(inputs, trace=False, **kw):
    from concourse.bass_utils import run_bass_kernel_spmd

    in_maps = _prep(inputs)
    nc = _get_nc()
    r = run_bass_kernel_spmd(nc, in_maps, core_ids=list(range(NCORES)),
                             trace=trace, **kw)
    bo = np.asarray(inputs["bo"], np.float32)
    out = np.zeros((B, Q, D), np.float32)
    for b in range(B):
        out[b] = (r.results[2 * b]["out"].astype(np.float32)
                  + r.results[2 * b + 1]["out"].astype(np.float32) + bo)
    return out, r


def kernel(**inputs):
    out, _ = _run(inputs, trace=False)
    return out
